# revision 1
# baseline (speedup 1.0000x reference)
"""Causal transformer block (B=2,S=2048,D=1024,H=16) on 8 trn2 NeuronCores.

Strategy: tensor-parallel attention over heads (2 heads/core) + token-parallel
MLP (512 tokens/core), glued by a single small AllToAll (1MB/core, bf16) that
re-shards the attention output from head-major to token-major.

vs the f32r baseline: all matmul operands are bf16 (same PE rate at wide
moving dims, but 2x transposes, 2x DMA, halved SBUF), V is computed directly
token-major (no post-transpose), all matmul biases are folded into PSUM
evacuations (tensor_scalar / activation-bias) or rank-1 ones-row accumulation
steps, exp() is batched two k-tiles at a time to halve activation-engine fixed
costs, the act-table only switches 4x per pass (Sqrt/Exp/Sqrt/Gelu), and the
phase-C weights (wp, w1) are prefetched into SBUF during attention so the
post-collective region is pure compute.  PSUM evacuations run on DVE/Act only
(GPSIMD cannot touch PSUM); LN normalize runs on GPSIMD.
"""
import numpy as np
from contextlib import ExitStack

import concourse.bass as bass
import concourse.bacc as bacc
import concourse.tile as tile
from concourse import mybir
from concourse.bass_utils import run_bass_kernel_spmd

f32 = mybir.dt.float32
f32r = mybir.dt.float32r
bf16 = mybir.dt.bfloat16
AF = mybir.ActivationFunctionType
ALU = mybir.AluOpType

B, S, D, H = 2, 2048, 1024, 16
HD, FF, NCORES = 64, 4096, 8
TOK = B * S            # 4096 total tokens
CHK = TOK // NCORES    # 512 tokens per core
D8 = D // 128          # 8 contraction tiles over D
NF = FF // 128         # 32 tiles over FF
NB = TOK // 512        # 8 token blocks of 512
EPS = 1e-5
SCALE = 1.0 / float(np.sqrt(HD))


def build(loops=1):
    nc = bacc.Bacc(None, num_devices=NCORES)

    x_h = nc.declare_dram_parameter("x", [TOK, D], bf16, isOutput=False)
    xc_h = nc.declare_dram_parameter("xc", [CHK, D], f32, isOutput=False)
    wq_h = nc.declare_dram_parameter("wq", [128, D8, 128], bf16, isOutput=False)
    wk_h = nc.declare_dram_parameter("wk", [128, D8, 128], bf16, isOutput=False)
    wv_h = nc.declare_dram_parameter("wv", [128, D8, 128], bf16, isOutput=False)
    bq_h = nc.declare_dram_parameter("bq", [128, 1], f32, isOutput=False)
    bk_h = nc.declare_dram_parameter("bk", [128, 1], f32, isOutput=False)
    bv_h = nc.declare_dram_parameter("bv", [1, 128], bf16, isOutput=False)
    wp_h = nc.declare_dram_parameter("wp", [128, D8, D], bf16, isOutput=False)
    bp_h = nc.declare_dram_parameter("bp", [1, D], bf16, isOutput=False)
    w1_h = nc.declare_dram_parameter("w1", [NF, 128, D8, 128], bf16, isOutput=False)
    b1_h = nc.declare_dram_parameter("b1", [128, NF], f32, isOutput=False)
    w2_h = nc.declare_dram_parameter("w2", [NF, 128, D], bf16, isOutput=False)
    b2_h = nc.declare_dram_parameter("b2", [1, D], bf16, isOutput=False)
    mask_h = nc.declare_dram_parameter("mask", [128, 128], f32, isOutput=False)
    id_h = nc.declare_dram_parameter("ident", [128, 128], bf16, isOutput=False)
    ones_h = nc.declare_dram_parameter("ones", [1, 512], bf16, isOutput=False)
    onesd_h = nc.declare_dram_parameter("onesd", [1, HD], f32, isOutput=False)
    onesv_h = nc.declare_dram_parameter("onesv", [128, TOK // 128, 2, HD], bf16,
                                        isOutput=False)
    out_h = nc.declare_dram_parameter("out", [CHK, D], f32, isOutput=True)

    with tile.TileContext(nc) as tc, ExitStack() as top:
        const = top.enter_context(tc.tile_pool(name="const", bufs=1))
        dram = top.enter_context(tc.tile_pool(name="dramp", bufs=1, space="DRAM"))

        ident = const.tile([128, 128], bf16)
        nc.gpsimd.dma_start(out=ident[:], in_=id_h[:])
        mask = const.tile([128, 128], f32)
        nc.gpsimd.dma_start(out=mask[:], in_=mask_h[:])
        eps_sb = const.tile([128, 1], f32)
        nc.vector.memset(eps_sb[:], EPS)
        ones = const.tile([1, 512], bf16)
        nc.gpsimd.dma_start(out=ones[:], in_=ones_h[:])
        bq_sb = const.tile([128, 1], f32)
        nc.gpsimd.dma_start(out=bq_sb[:], in_=bq_h[:])
        bk_sb = const.tile([128, 1], f32)
        nc.gpsimd.dma_start(out=bk_sb[:], in_=bk_h[:])
        bv_sb = const.tile([1, 128], bf16)
        nc.gpsimd.dma_start(out=bv_sb[:], in_=bv_h[:])
        bp_sb = const.tile([1, D], bf16)
        nc.gpsimd.dma_start(out=bp_sb[:], in_=bp_h[:])
        b1_sb = const.tile([128, NF], f32)
        nc.gpsimd.dma_start(out=b1_sb[:], in_=b1_h[:])
        b2_sb = const.tile([1, D], bf16)
        nc.gpsimd.dma_start(out=b2_sb[:], in_=b2_h[:])

        # AllToAll per head half: Ic[chunk, my-64-attn-rows, 512 toks] ->
        # Oc[src, 64 rows, my 512 toks]
        Ic0 = dram.tile([NCORES, HD, CHK], bf16)
        Oc0 = dram.tile([NCORES, HD, CHK], bf16)
        Ic1 = dram.tile([NCORES, HD, CHK], bf16)
        Oc1 = dram.tile([NCORES, HD, CHK], bf16)

        def phase_a1(wA, QT, KT, V):
            """LN1 + QKV^T for all 4096 tokens (this core's 128 qkv rows)."""
            wq_sb = wA.tile([128, D8, 128], bf16)
            nc.gpsimd.dma_start(out=wq_sb[:], in_=wq_h[:])
            wk_sb = wA.tile([128, D8, 128], bf16)
            nc.gpsimd.dma_start(out=wk_sb[:], in_=wk_h[:])
            wv_sb = wA.tile([128, D8, 128], bf16)
            nc.gpsimd.dma_start(out=wv_sb[:], in_=wv_h[:])
            nc.gpsimd.dma_start(out=V[:, :, :, HD:2 * HD], in_=onesv_h[:])

            with ExitStack() as A1:
                lnp = A1.enter_context(tc.tile_pool(name="lnp", bufs=4))
                hTp = A1.enter_context(tc.tile_pool(name="hTp", bufs=2))
                ps_tr = A1.enter_context(
                    tc.tile_pool(name="ps_tr", bufs=2, space="PSUM"))
                ps_mm = A1.enter_context(
                    tc.tile_pool(name="ps_mm", bufs=2, space="PSUM"))
                ps_v = A1.enter_context(
                    tc.tile_pool(name="ps_v", bufs=2, space="PSUM"))

                for blk in range(NB):
                    hTb = hTp.tile([128, D8, 512], bf16, tag="hTb")
                    for t4 in range(4):
                        tt = blk * 4 + t4
                        xt = lnp.tile([128, D], bf16, tag="xt")
                        nc.sync.dma_start(out=xt[:], in_=x_h[tt * 128:(tt + 1) * 128, :])
                        st = lnp.tile([128, 2, 6], f32, tag="st")
                        xv = xt[:].rearrange("p (s d) -> p s d", s=2)
                        nc.vector.bn_stats(out=st[:, 0, :], in_=xv[:, 0, :])
                        nc.vector.bn_stats(out=st[:, 1, :], in_=xv[:, 1, :])
                        mv = lnp.tile([128, 2], f32, tag="mv")
                        nc.vector.bn_aggr(out=mv[:], in_=st[:])
                        nc.scalar.activation(out=mv[:, 1:2], in_=mv[:, 1:2],
                                             func=AF.Sqrt, bias=eps_sb[:])
                        nc.vector.reciprocal(out=mv[:, 1:2], in_=mv[:, 1:2])
                        yt = lnp.tile([128, D], bf16, tag="yt")
                        nc.gpsimd.tensor_scalar(out=yt[:], in0=xt[:],
                                                scalar1=mv[:, 0:1], scalar2=mv[:, 1:2],
                                                op0=ALU.subtract, op1=ALU.mult)
                        tp = ps_tr.tile([128, D8, 128], bf16, tag="tp")
                        for a in range(D8):
                            nc.tensor.transpose(tp[:, a, :], yt[:, a * 128:(a + 1) * 128],
                                                ident[:])
                        nc.scalar.copy(out=hTb[:, :, t4 * 128:(t4 + 1) * 128],
                                       in_=tp[:])

                    for w_sb, b_sb, dst in ((wq_sb, bq_sb, QT), (wk_sb, bk_sb, KT)):
                        ps = ps_mm.tile([128, 512], f32, tag="qkps")
                        for a in range(D8):
                            nc.tensor.matmul(ps[:], w_sb[:, a, :], hTb[:, a, :],
                                             start=(a == 0), stop=(a == D8 - 1))
                        nc.vector.tensor_scalar(
                            out=dst[:, blk * 512:(blk + 1) * 512], in0=ps[:],
                            scalar1=b_sb[:], scalar2=None, op0=ALU.add)
                    for tsub in range(4):
                        psv = ps_v.tile([128, 128], f32, tag="psv")
                        for a in range(D8):
                            nc.tensor.matmul(psv[:],
                                             hTb[:, a, tsub * 128:(tsub + 1) * 128],
                                             wv_sb[:, a, :],
                                             start=(a == 0), stop=False)
                        nc.tensor.matmul(psv[:], ones[:, 0:128], bv_sb[:],
                                         start=False, stop=True)
                        nc.vector.tensor_copy(
                            out=V[:, blk * 4 + tsub, :, 0:HD],
                            in_=psv[:].rearrange("p (h d) -> p h d", h=2))

        def phase_a2(pools, h2, Ic, QT, KT, V):
            """Causal attention for one head (both batches); exp batched 2 k-tiles."""
            ptp, smp, aop, ps_s, ps_av = pools
            for bh in range(B):
                ro = h2 * HD
                for j in range(4):          # query blocks of 512
                    q0 = bh * S + j * 512
                    nkt = 4 * (j + 1)
                    av = ps_av.tile([128, 512], f32, tag="av")
                    for kp in range(nkt // 2):
                        sp = ps_s.tile([128, 1024], f32, tag="sp")
                        pt = ptp.tile([128, 1024], bf16, tag="pt")
                        r = 0
                        info = []
                        for kt in (2 * kp, 2 * kp + 1):
                            k0 = bh * S + kt * 128
                            ofs = max(0, 128 * kt - 512 * j)
                            w = 512 - ofs
                            nc.tensor.matmul(
                                sp[:, r:r + w], KT[ro:ro + HD, k0:k0 + 128],
                                QT[ro:ro + HD, q0 + ofs:q0 + 512],
                                start=True, stop=True)
                            if kt >= 4 * j:
                                nc.vector.tensor_add(sp[:, r:r + 128],
                                                     sp[:, r:r + 128], mask[:])
                            info.append((kt, r, ofs, w))
                            r += w
                        nc.scalar.activation(out=pt[:, 0:r], in_=sp[:, 0:r],
                                             func=AF.Exp, scale=SCALE)
                        for kt, r0, ofs, w in info:
                            g = bh * (S // 128) + kt
                            nc.tensor.matmul(av[:, ofs:512],
                                             V[:, g, h2, :],
                                             pt[:, r0:r0 + w],
                                             start=(kt == 0), stop=(kt == nkt - 1))
                    # av rows 64:128 all hold the softmax denominator (64
                    # ones-columns in V), so normalize is pure elementwise
                    rbc = aop.tile([HD, 512], f32, tag="rbc")
                    nc.vector.reciprocal(out=rbc[:], in_=av[HD:2 * HD, :])
                    ao = aop.tile([HD, 512], bf16, tag="ao")
                    nc.vector.tensor_mul(ao[:], av[0:HD, :], rbc[:])
                    chunk = bh * 4 + j
                    nc.sync.dma_start(out=Ic[chunk], in_=ao[:])

        def phase_c(rB, wp_sb):
            """proj + residual + LN2 + MLP for this core's 512 tokens."""
            Oc_sb = rB.tile([128, NCORES, CHK], bf16)
            nc.scalar.dma_start(out=Oc_sb[0:HD, :, :],
                                in_=Oc0[:].rearrange("i p t -> p i t"))
            nc.scalar.dma_start(out=Oc_sb[HD:128, :, :],
                                in_=Oc1[:].rearrange("i p t -> p i t"))
            x2_sb = rB.tile([128, 4, D], f32)
            y2T = rB.tile([128, D8, CHK], bf16)
            g1T = rB.tile([128, NF, CHK], bf16)

            with ExitStack() as C:
                w2sp = C.enter_context(tc.tile_pool(name="w2s", bufs=3))
                with ExitStack() as C1:
                    lnp2 = C1.enter_context(tc.tile_pool(name="lnp2", bufs=2))
                    w1s = C1.enter_context(tc.tile_pool(name="w1s", bufs=3))
                    ps_p = C1.enter_context(
                        tc.tile_pool(name="ps_p", bufs=2, space="PSUM"))
                    ps_t2 = C1.enter_context(
                        tc.tile_pool(name="ps_t2", bufs=2, space="PSUM"))
                    ps_f1 = C1.enter_context(
                        tc.tile_pool(name="ps_f1", bufs=2, space="PSUM"))

                    for t4 in range(4):
                        xct = lnp2.tile([128, D], f32, tag="xct")
                        nc.sync.dma_start(out=xct[:], in_=xc_h[t4 * 128:(t4 + 1) * 128, :])
                        for dc in range(2):
                            ps = ps_p.tile([128, 512], f32, tag="pp")
                            for a in range(D8):
                                nc.tensor.matmul(ps[:], Oc_sb[:, a, t4 * 128:(t4 + 1) * 128],
                                                 wp_sb[:, a, dc * 512:(dc + 1) * 512],
                                                 start=(a == 0), stop=False)
                            nc.tensor.matmul(ps[:], ones[:, 0:128],
                                             bp_sb[:, dc * 512:(dc + 1) * 512],
                                             start=False, stop=True)
                            nc.vector.tensor_add(x2_sb[:, t4, dc * 512:(dc + 1) * 512],
                                                 ps[:], xct[:, dc * 512:(dc + 1) * 512])
                        st2 = lnp2.tile([128, 2, 6], f32, tag="st2")
                        x2v = x2_sb[:, t4, :].rearrange("p (s d) -> p s d", s=2)
                        nc.vector.bn_stats(out=st2[:, 0, :], in_=x2v[:, 0, :])
                        nc.vector.bn_stats(out=st2[:, 1, :], in_=x2v[:, 1, :])
                        mv2 = lnp2.tile([128, 2], f32, tag="mv2")
                        nc.vector.bn_aggr(out=mv2[:], in_=st2[:])
                        nc.scalar.activation(out=mv2[:, 1:2], in_=mv2[:, 1:2],
                                             func=AF.Sqrt, bias=eps_sb[:])
                        nc.vector.reciprocal(out=mv2[:, 1:2], in_=mv2[:, 1:2])
                        y2 = lnp2.tile([128, D], bf16, tag="y2")
                        nc.gpsimd.tensor_scalar(out=y2[:], in0=x2_sb[:, t4, :],
                                                scalar1=mv2[:, 0:1], scalar2=mv2[:, 1:2],
                                                op0=ALU.subtract, op1=ALU.mult)
                        tp2 = ps_t2.tile([128, D8, 128], bf16, tag="t2")
                        for a in range(D8):
                            nc.tensor.transpose(tp2[:, a, :], y2[:, a * 128:(a + 1) * 128],
                                                ident[:])
                        nc.scalar.copy(out=y2T[:, :, t4 * 128:(t4 + 1) * 128], in_=tp2[:])

                    # fc1 + bias + gelu (fused on act) -> g1T resident
                    for ff in range(NF):
                        w1t = w1s.tile([128, D8, 128], bf16, tag="w1t")
                        nc.sync.dma_start(out=w1t[:], in_=w1_h[ff])
                        ps = ps_f1.tile([128, 512], f32, tag="f1")
                        for a in range(D8):
                            nc.tensor.matmul(ps[:], w1t[:, a, :], y2T[:, a, :],
                                             start=(a == 0), stop=(a == D8 - 1))
                        nc.scalar.activation(out=g1T[:, ff, :], in_=ps[:], func=AF.Gelu,
                                             bias=b1_sb[:, ff:ff + 1])

                # fc2: 8 psum accumulators (4 token tiles x 2 column halves)
                ps_f2 = C.enter_context(tc.tile_pool(name="ps_f2", bufs=1, space="PSUM"))
                outp = C.enter_context(tc.tile_pool(name="outp", bufs=2))
                accs = [ps_f2.tile([128, 512], f32, name=f"acc{i}", tag=f"acc{i}")
                        for i in range(8)]
                for ff in range(NF):
                    w2t = w2sp.tile([128, D], bf16, tag="w2t")
                    nc.sync.dma_start(out=w2t[:], in_=w2_h[ff])
                    for t4 in range(4):
                        for dc in range(2):
                            nc.tensor.matmul(accs[t4 * 2 + dc][:],
                                             g1T[:, ff, t4 * 128:(t4 + 1) * 128],
                                             w2t[:, dc * 512:(dc + 1) * 512],
                                             start=(ff == 0), stop=False)
                for t4 in range(4):
                    ot = outp.tile([128, D], f32, tag="ot")
                    for dc in range(2):
                        i = t4 * 2 + dc
                        nc.tensor.matmul(accs[i][:], ones[:, 0:128],
                                         b2_sb[:, dc * 512:(dc + 1) * 512],
                                         start=False, stop=True)
                        nc.vector.tensor_add(ot[:, dc * 512:(dc + 1) * 512], accs[i][:],
                                             x2_sb[:, t4, dc * 512:(dc + 1) * 512])
                    nc.sync.dma_start(out=out_h[t4 * 128:(t4 + 1) * 128, :], in_=ot[:])

        def one_pass():
            with ExitStack() as P:
                rB = P.enter_context(tc.tile_pool(name="rB", bufs=1))
                with ExitStack() as A:
                    wA = A.enter_context(tc.tile_pool(name="wA", bufs=1))
                    QT = wA.tile([128, TOK], bf16)   # rows: (h2, hd)
                    KT = wA.tile([128, TOK], bf16)
                    # token-major V per 128-tok group g, per head:
                    # 64 dims + 64 ones columns (denominator rows of av)
                    V = wA.tile([128, TOK // 128, 2, 2 * HD], bf16)
                    phase_a1(wA, QT, KT, V)
                    # prefetch the proj weight during attention (act dma queue)
                    wp_sb = rB.tile([128, D8, D], bf16)
                    nc.scalar.dma_start(out=wp_sb[:], in_=wp_h[:])
                    pools = (
                        A.enter_context(tc.tile_pool(name="ptp", bufs=3)),
                        A.enter_context(tc.tile_pool(name="smp", bufs=2)),
                        A.enter_context(tc.tile_pool(name="aop", bufs=2)),
                        A.enter_context(tc.tile_pool(name="ps_s", bufs=3, space="PSUM")),
                        A.enter_context(tc.tile_pool(name="ps_av", bufs=2, space="PSUM")),
                    )
                    # head 0 of both batches, then its half-AllToAll overlaps
                    # head 1's attention; second half-AllToAll after.
                    phase_a2(pools, 0, Ic0, QT, KT, V)
                    nc.gpsimd.collective_compute(
                        "AllToAll", ALU.bypass,
                        replica_groups=[list(range(NCORES))],
                        ins=[Ic0[:]], outs=[Oc0[:]],
                    )
                    phase_a2(pools, 1, Ic1, QT, KT, V)

                # ----------- AllToAll (head 1): head-major -> token-major --------
                nc.gpsimd.collective_compute(
                    "AllToAll", ALU.bypass,
                    replica_groups=[list(range(NCORES))],
                    ins=[Ic1[:]], outs=[Oc1[:]],
                )

                phase_c(rB, wp_sb)

        for _ in range(loops):
            one_pass()
    nc.finalize()
    return nc


_NC_CACHE = []
LAST = None


def _get_nc():
    if not _NC_CACHE:
        _NC_CACHE.append(build())
    return _NC_CACHE[0]


def prepare_in_maps(inputs):
    from ml_dtypes import bfloat16

    f = np.float32
    x = np.asarray(inputs["x"], f).reshape(TOK, D)
    ln1_g = np.asarray(inputs["ln1_g"], np.float64)
    ln1_b = np.asarray(inputs["ln1_b"], np.float64)
    ln2_g = np.asarray(inputs["ln2_g"], np.float64)
    ln2_b = np.asarray(inputs["ln2_b"], np.float64)
    w_qkv = np.asarray(inputs["w_qkv"], np.float64)
    b_qkv = np.asarray(inputs["b_qkv"], np.float64)
    w_fc1 = np.asarray(inputs["w_fc1"], np.float64)
    b_fc1 = np.asarray(inputs["b_fc1"], np.float64)

    w_eff = (w_qkv * ln1_g[:, None]).astype(f)
    b_eff = (b_qkv + ln1_b @ w_qkv).astype(f)
    w1_eff = (w_fc1 * ln2_g[:, None]).astype(f)
    b1_eff = (b_fc1 + ln2_b @ w_fc1).astype(f)
    wp = np.asarray(inputs["w_proj"], f)
    bp = np.asarray(inputs["b_proj"], f).reshape(1, D)
    w2 = np.asarray(inputs["w_fc2"], f)
    b2 = np.asarray(inputs["b_fc2"], f).reshape(1, D)

    bf = lambda a: np.ascontiguousarray(a).astype(bfloat16)
    # [D, N] weight -> stationary layout [p, a, cols], feature f = a*128+p
    stat = lambda w: np.ascontiguousarray(
        w.reshape(D8, 128, w.shape[1]).transpose(1, 0, 2))

    x_bf = bf(x)
    wp_r = bf(stat(wp))                                     # [128, 8, 1024]
    w1_r = bf(np.ascontiguousarray(                         # [32, 128, 8, 128]
        w1_eff.reshape(D8, 128, NF, 128).transpose(2, 1, 0, 3)))
    w2_r = bf(w2.reshape(NF, 128, D))                       # [32, 128, 1024]
    b1_c = np.ascontiguousarray(b1_eff.reshape(NF, 128).T, dtype=f)  # [128, 32]

    # additive causal mask for the diagonal 128x128 block of scores[k, q]:
    # valid (zero) where q >= k, else -1e9
    mask = np.full((128, 128), -1e9, f)
    for i in range(128):
        mask[i, i:] = 0.0
    ident = np.eye(128, dtype=bfloat16)

    in_maps = []
    for c in range(NCORES):
        cs = slice(128 * c, 128 * (c + 1))
        in_maps.append({
            "x": x_bf,
            "xc": np.ascontiguousarray(x[CHK * c:CHK * (c + 1)]),
            "wq": bf(stat(w_eff[:, 0 * D:1 * D][:, cs])),
            "wk": bf(stat(w_eff[:, 1 * D:2 * D][:, cs])),
            "wv": bf(stat(w_eff[:, 2 * D:3 * D][:, cs])),
            "bq": np.ascontiguousarray(b_eff[0 * D:1 * D][cs].reshape(128, 1), dtype=f),
            "bk": np.ascontiguousarray(b_eff[1 * D:2 * D][cs].reshape(128, 1), dtype=f),
            "bv": bf(b_eff[2 * D:3 * D][cs].reshape(1, 128)),
            "wp": wp_r, "bp": bf(bp),
            "w1": w1_r, "b1": b1_c,
            "w2": w2_r, "b2": bf(b2),
            "mask": mask, "ident": ident,
            "ones": np.ones((1, 512), bfloat16),
            "onesd": np.ones((1, HD), f),
            "onesv": np.ones((128, TOK // 128, 2, HD), bfloat16),
        })
    return in_maps


def kernel(**inputs):
    global LAST
    in_maps = prepare_in_maps(inputs)
    nc = _get_nc()
    res = run_bass_kernel_spmd(nc, in_maps, list(range(NCORES)))
    LAST = res
    out = np.concatenate([res.results[c]["out"] for c in range(NCORES)], axis=0)
    return out.reshape(B, S, D).astype(np.float32, copy=False)



# revision 5
# speedup vs baseline: 1.8486x; 1.8486x over previous
"""Causal transformer block (B=2,S=2048,D=1024,H=16) on 8 trn2 NeuronCores.

Strategy: tensor-parallel attention over heads (2 heads/core) + token-parallel
MLP (512 tokens/core), glued by a single small AllToAll (1MB/core, bf16) that
re-shards the attention output from head-major to token-major.

LayerNorm is folded into the matmuls: instead of an elementwise
(x-m)*rstd pass, the PE transposes raw x through diag(rstd) (the transpose IS
a matmul, so the per-token column scale is free), and the -mean*rstd term is
a rank-1 PSUM accumulation against precomputed negated weight column sums.
All matmul operands are bf16, V is computed directly token-major, matmul
biases are folded into PSUM evacuations or rank-1 ones-row accumulation
steps, exp() is batched two k-tiles at a time, and the phase-C weights
(wp, w1) are prefetched into SBUF during attention.
"""
import numpy as np
from contextlib import ExitStack

import concourse.bass as bass
import concourse.bacc as bacc
import concourse.tile as tile
from concourse import mybir
from concourse.bass_utils import run_bass_kernel_spmd

f32 = mybir.dt.float32
f32r = mybir.dt.float32r
bf16 = mybir.dt.bfloat16
AF = mybir.ActivationFunctionType
ALU = mybir.AluOpType

B, S, D, H = 2, 2048, 1024, 16
HD, FF, NCORES = 64, 4096, 8
TOK = B * S            # 4096 total tokens
CHK = TOK // NCORES    # 512 tokens per core
D8 = D // 128          # 8 contraction tiles over D
NF = FF // 128         # 32 tiles over FF
NB = TOK // 512        # 8 token blocks of 512
EPS = 1e-5
SCALE = 1.0 / float(np.sqrt(HD))


def build(loops=1):
    nc = bacc.Bacc(None, num_devices=NCORES)

    x_h = nc.declare_dram_parameter("x", [TOK, D], bf16, isOutput=False)
    xc_h = nc.declare_dram_parameter("xc", [CHK, D], f32, isOutput=False)
    wq_h = nc.declare_dram_parameter("wq", [128, D8, 128], bf16, isOutput=False)
    wk_h = nc.declare_dram_parameter("wk", [128, D8, 128], bf16, isOutput=False)
    wv_h = nc.declare_dram_parameter("wv", [128, D8, 128], bf16, isOutput=False)
    bq_h = nc.declare_dram_parameter("bq", [128, 1], f32, isOutput=False)
    bk_h = nc.declare_dram_parameter("bk", [128, 1], f32, isOutput=False)
    bv_h = nc.declare_dram_parameter("bv", [1, 128], bf16, isOutput=False)
    nsq_h = nc.declare_dram_parameter("nsq", [1, 128], bf16, isOutput=False)
    nsk_h = nc.declare_dram_parameter("nsk", [1, 128], bf16, isOutput=False)
    nsv_h = nc.declare_dram_parameter("nsv", [1, 128], bf16, isOutput=False)
    ns1_h = nc.declare_dram_parameter("ns1", [1, FF], bf16, isOutput=False)
    wp_h = nc.declare_dram_parameter("wp", [128, D8, D], bf16, isOutput=False)
    bp_h = nc.declare_dram_parameter("bp", [1, D], bf16, isOutput=False)
    w1_h = nc.declare_dram_parameter("w1", [NF, 128, D8, 128], bf16, isOutput=False)
    b1_h = nc.declare_dram_parameter("b1", [128, NF], f32, isOutput=False)
    w2_h = nc.declare_dram_parameter("w2", [NF, 128, D], bf16, isOutput=False)
    b2_h = nc.declare_dram_parameter("b2", [1, D], bf16, isOutput=False)
    mask_h = nc.declare_dram_parameter("mask", [128, 128], f32, isOutput=False)
    id_h = nc.declare_dram_parameter("ident", [128, 128], bf16, isOutput=False)
    ones_h = nc.declare_dram_parameter("ones", [1, 512], bf16, isOutput=False)
    onesv_h = nc.declare_dram_parameter("onesv", [128, TOK // 128, 2, HD], bf16,
                                        isOutput=False)
    out_h = nc.declare_dram_parameter("out", [CHK, D], f32, isOutput=True)

    with tile.TileContext(nc) as tc, ExitStack() as top:
        const = top.enter_context(tc.tile_pool(name="const", bufs=1))
        dram = top.enter_context(tc.tile_pool(name="dramp", bufs=1, space="DRAM"))

        ident = const.tile([128, 128], bf16)
        nc.gpsimd.dma_start(out=ident[:], in_=id_h[:])
        mask = const.tile([128, 128], f32)
        nc.gpsimd.dma_start(out=mask[:], in_=mask_h[:])
        eps_sb = const.tile([128, 1], f32)
        nc.vector.memset(eps_sb[:], EPS)
        ones = const.tile([1, 512], bf16)
        nc.gpsimd.dma_start(out=ones[:], in_=ones_h[:])
        bq_sb = const.tile([128, 1], f32)
        nc.gpsimd.dma_start(out=bq_sb[:], in_=bq_h[:])
        bk_sb = const.tile([128, 1], f32)
        nc.gpsimd.dma_start(out=bk_sb[:], in_=bk_h[:])
        bv_sb = const.tile([1, 128], bf16)
        nc.gpsimd.dma_start(out=bv_sb[:], in_=bv_h[:])
        nsq_sb = const.tile([1, 128], bf16)
        nc.gpsimd.dma_start(out=nsq_sb[:], in_=nsq_h[:])
        nsk_sb = const.tile([1, 128], bf16)
        nc.gpsimd.dma_start(out=nsk_sb[:], in_=nsk_h[:])
        nsv_sb = const.tile([1, 128], bf16)
        nc.gpsimd.dma_start(out=nsv_sb[:], in_=nsv_h[:])
        ns1_sb = const.tile([1, FF], bf16)
        nc.gpsimd.dma_start(out=ns1_sb[:], in_=ns1_h[:])
        bp_sb = const.tile([1, D], bf16)
        nc.gpsimd.dma_start(out=bp_sb[:], in_=bp_h[:])
        b1_sb = const.tile([128, NF], f32)
        nc.gpsimd.dma_start(out=b1_sb[:], in_=b1_h[:])
        b2_sb = const.tile([1, D], bf16)
        nc.gpsimd.dma_start(out=b2_sb[:], in_=b2_h[:])

        # AllToAll per head half: Ic[chunk, my-64-attn-rows, 512 toks] ->
        # Oc[src, 64 rows, my 512 toks]
        Ic0 = dram.tile([NCORES, HD, CHK], bf16)
        Oc0 = dram.tile([NCORES, HD, CHK], bf16)
        Ic1 = dram.tile([NCORES, HD, CHK], bf16)
        Oc1 = dram.tile([NCORES, HD, CHK], bf16)

        def phase_a1(wA, QT, KT, V):
            """LN1 (folded) + QKV^T for all 4096 tokens (this core's 128 rows)."""
            wq_sb = wA.tile([128, D8, 128], bf16)
            nc.gpsimd.dma_start(out=wq_sb[:], in_=wq_h[:])
            wk_sb = wA.tile([128, D8, 128], bf16)
            nc.gpsimd.dma_start(out=wk_sb[:], in_=wk_h[:])
            wv_sb = wA.tile([128, D8, 128], bf16)
            nc.gpsimd.dma_start(out=wv_sb[:], in_=wv_h[:])
            nc.gpsimd.dma_start(out=V[:, :, :, HD:2 * HD], in_=onesv_h[:])

            with ExitStack() as A1:
                lnp = A1.enter_context(tc.tile_pool(name="lnp", bufs=4))
                hTp = A1.enter_context(tc.tile_pool(name="hTp", bufs=2))
                ps_tr = A1.enter_context(
                    tc.tile_pool(name="ps_tr", bufs=2, space="PSUM"))
                ps_mm = A1.enter_context(
                    tc.tile_pool(name="ps_mm", bufs=2, space="PSUM"))
                ps_v = A1.enter_context(
                    tc.tile_pool(name="ps_v", bufs=2, space="PSUM"))

                for blk in range(NB):
                    hTb = hTp.tile([128, D8, 512], bf16, tag="hTb")
                    mrT_ps = ps_tr.tile([1, 512], bf16, tag="mrT")
                    for t4 in range(4):
                        tt = blk * 4 + t4
                        xt = lnp.tile([128, D], bf16, tag="xt")
                        nc.sync.dma_start(out=xt[:], in_=x_h[tt * 128:(tt + 1) * 128, :])
                        st = lnp.tile([128, 2, 6], f32, tag="st")
                        xv = xt[:].rearrange("p (s d) -> p s d", s=2)
                        nc.vector.bn_stats(out=st[:, 0, :], in_=xv[:, 0, :])
                        nc.vector.bn_stats(out=st[:, 1, :], in_=xv[:, 1, :])
                        mv = lnp.tile([128, 2], f32, tag="mv")
                        nc.vector.bn_aggr(out=mv[:], in_=st[:])
                        nc.scalar.activation(out=mv[:, 1:2], in_=mv[:, 1:2],
                                             func=AF.Sqrt, bias=eps_sb[:])
                        nc.vector.reciprocal(out=mv[:, 1:2], in_=mv[:, 1:2])
                        # mr = mean * rstd (bf16), diag(rstd) for the scaled
                        # transpose (a real matmul: transpose-mode ignores the
                        # moving operand, so it can't apply the scale)
                        mr = lnp.tile([128, 1], bf16, tag="mr")
                        nc.vector.tensor_mul(mr[:], mv[:, 0:1], mv[:, 1:2])
                        dgr = lnp.tile([128, 128], bf16, tag="dgr")
                        nc.vector.tensor_scalar(
                            out=dgr[:], in0=ident[:], scalar1=mv[:, 1:2],
                            scalar2=None, op0=ALU.mult)
                        for h in range(2):
                            tp = ps_tr.tile([128, 4, 128], f32, tag="tp")
                            for a in range(4):
                                nc.tensor.matmul(
                                    tp[:, a, :],
                                    xt[:, (h * 4 + a) * 128:(h * 4 + a + 1) * 128],
                                    dgr[:], start=True, stop=True)
                            nc.scalar.copy(
                                out=hTb[:, h * 4:(h + 1) * 4,
                                        t4 * 128:(t4 + 1) * 128],
                                in_=tp[:])
                        nc.tensor.transpose(mrT_ps[:, t4 * 128:(t4 + 1) * 128],
                                            mr[:], ident[:])
                    mrT = lnp.tile([1, 512], bf16, tag="mrT_sb")
                    nc.scalar.copy(out=mrT[:], in_=mrT_ps[:])

                    for w_sb, b_sb, ns_sb, dst in (
                            (wq_sb, bq_sb, nsq_sb, QT), (wk_sb, bk_sb, nsk_sb, KT)):
                        ps = ps_mm.tile([128, 512], f32, tag="qkps")
                        for a in range(D8):
                            nc.tensor.matmul(ps[:], w_sb[:, a, :], hTb[:, a, :],
                                             start=(a == 0), stop=False)
                        nc.tensor.matmul(ps[:], ns_sb[:], mrT[:],
                                         start=False, stop=True)
                        nc.vector.tensor_scalar(
                            out=dst[:, blk * 512:(blk + 1) * 512], in0=ps[:],
                            scalar1=b_sb[:], scalar2=None, op0=ALU.add)
                    for tsub in range(4):
                        psv = ps_v.tile([128, 128], f32, tag="psv")
                        for a in range(D8):
                            nc.tensor.matmul(psv[:],
                                             hTb[:, a, tsub * 128:(tsub + 1) * 128],
                                             wv_sb[:, a, :],
                                             start=(a == 0), stop=False)
                        nc.tensor.matmul(psv[:],
                                         mrT[:, tsub * 128:(tsub + 1) * 128],
                                         nsv_sb[:], start=False, stop=False)
                        nc.tensor.matmul(psv[:], ones[:, 0:128], bv_sb[:],
                                         start=False, stop=True)
                        nc.vector.tensor_copy(
                            out=V[:, blk * 4 + tsub, :, 0:HD],
                            in_=psv[:].rearrange("p (h d) -> p h d", h=2))

        def phase_a2(pools, h2, Ic, QT, KT, V):
            """Causal attention for one head (both batches); exp batched 2 k-tiles."""
            ptp, smp, aop, ps_s, ps_av = pools
            for bh in range(B):
                ro = h2 * HD
                for j in range(4):          # query blocks of 512
                    q0 = bh * S + j * 512
                    nkt = 4 * (j + 1)
                    av = ps_av.tile([128, 512], f32, tag="av")
                    for kp in range(nkt // 2):
                        sp = ps_s.tile([128, 1024], f32, tag="sp")
                        pt = ptp.tile([128, 1024], bf16, tag="pt")
                        r = 0
                        info = []
                        for kt in (2 * kp, 2 * kp + 1):
                            k0 = bh * S + kt * 128
                            ofs = max(0, 128 * kt - 512 * j)
                            w = 512 - ofs
                            nc.tensor.matmul(
                                sp[:, r:r + w], KT[ro:ro + HD, k0:k0 + 128],
                                QT[ro:ro + HD, q0 + ofs:q0 + 512],
                                start=True, stop=True)
                            if kt >= 4 * j:
                                nc.vector.tensor_add(sp[:, r:r + 128],
                                                     sp[:, r:r + 128], mask[:])
                            info.append((kt, r, ofs, w))
                            r += w
                        nc.scalar.activation(out=pt[:, 0:r], in_=sp[:, 0:r],
                                             func=AF.Exp, scale=SCALE)
                        for kt, r0, ofs, w in info:
                            g = bh * (S // 128) + kt
                            nc.tensor.matmul(av[:, ofs:512],
                                             V[:, g, h2, :],
                                             pt[:, r0:r0 + w],
                                             start=(kt == 0), stop=(kt == nkt - 1))
                    # av rows 64:128 all hold the softmax denominator (64
                    # ones-columns in V), so normalize is pure elementwise
                    rbc = aop.tile([HD, 512], f32, tag="rbc")
                    nc.vector.reciprocal(out=rbc[:], in_=av[HD:2 * HD, :])
                    ao = aop.tile([HD, 512], bf16, tag="ao")
                    nc.vector.tensor_mul(ao[:], av[0:HD, :], rbc[:])
                    chunk = bh * 4 + j
                    nc.sync.dma_start(out=Ic[chunk], in_=ao[:])

        def phase_c(rB, wp_sb):
            """proj + residual + LN2 (folded) + MLP for this core's 512 tokens."""
            Oc_sb = rB.tile([128, NCORES, CHK], bf16)
            nc.scalar.dma_start(out=Oc_sb[0:HD, :, :],
                                in_=Oc0[:].rearrange("i p t -> p i t"))
            nc.scalar.dma_start(out=Oc_sb[HD:128, :, :],
                                in_=Oc1[:].rearrange("i p t -> p i t"))
            x2_sb = rB.tile([128, 4, D], f32)
            y2T = rB.tile([128, D8, CHK], bf16)
            g1T = rB.tile([128, NF, CHK], bf16)

            with ExitStack() as C:
                w2sp = C.enter_context(tc.tile_pool(name="w2s", bufs=3))
                with ExitStack() as C1:
                    lnp2 = C1.enter_context(tc.tile_pool(name="lnp2", bufs=2))
                    w1s = C1.enter_context(tc.tile_pool(name="w1s", bufs=3))
                    ps_p = C1.enter_context(
                        tc.tile_pool(name="ps_p", bufs=2, space="PSUM"))
                    ps_t2 = C1.enter_context(
                        tc.tile_pool(name="ps_t2", bufs=2, space="PSUM"))
                    ps_f1 = C1.enter_context(
                        tc.tile_pool(name="ps_f1", bufs=2, space="PSUM"))

                    mr2T_ps = ps_t2.tile([1, 512], bf16, tag="mr2T")
                    for t4 in range(4):
                        xct = lnp2.tile([128, D], f32, tag="xct")
                        nc.sync.dma_start(out=xct[:], in_=xc_h[t4 * 128:(t4 + 1) * 128, :])
                        for dc in range(2):
                            ps = ps_p.tile([128, 512], f32, tag="pp")
                            for a in range(D8):
                                nc.tensor.matmul(ps[:], Oc_sb[:, a, t4 * 128:(t4 + 1) * 128],
                                                 wp_sb[:, a, dc * 512:(dc + 1) * 512],
                                                 start=(a == 0), stop=False)
                            nc.tensor.matmul(ps[:], ones[:, 0:128],
                                             bp_sb[:, dc * 512:(dc + 1) * 512],
                                             start=False, stop=True)
                            nc.vector.tensor_add(x2_sb[:, t4, dc * 512:(dc + 1) * 512],
                                                 ps[:], xct[:, dc * 512:(dc + 1) * 512])
                        st2 = lnp2.tile([128, 2, 6], f32, tag="st2")
                        x2v = x2_sb[:, t4, :].rearrange("p (s d) -> p s d", s=2)
                        nc.vector.bn_stats(out=st2[:, 0, :], in_=x2v[:, 0, :])
                        nc.vector.bn_stats(out=st2[:, 1, :], in_=x2v[:, 1, :])
                        mv2 = lnp2.tile([128, 2], f32, tag="mv2")
                        nc.vector.bn_aggr(out=mv2[:], in_=st2[:])
                        nc.scalar.activation(out=mv2[:, 1:2], in_=mv2[:, 1:2],
                                             func=AF.Sqrt, bias=eps_sb[:])
                        nc.vector.reciprocal(out=mv2[:, 1:2], in_=mv2[:, 1:2])
                        mr2 = lnp2.tile([128, 1], bf16, tag="mr2")
                        nc.vector.tensor_mul(mr2[:], mv2[:, 0:1], mv2[:, 1:2])
                        dgr2 = lnp2.tile([128, 128], bf16, tag="dgr2")
                        nc.vector.tensor_scalar(
                            out=dgr2[:], in0=ident[:], scalar1=mv2[:, 1:2],
                            scalar2=None, op0=ALU.mult)
                        x2b = lnp2.tile([128, D], bf16, tag="x2b")
                        nc.scalar.copy(out=x2b[:], in_=x2_sb[:, t4, :])
                        for h in range(2):
                            tp2 = ps_t2.tile([128, 4, 128], f32, tag="t2")
                            for a in range(4):
                                nc.tensor.matmul(
                                    tp2[:, a, :],
                                    x2b[:, (h * 4 + a) * 128:(h * 4 + a + 1) * 128],
                                    dgr2[:], start=True, stop=True)
                            nc.scalar.copy(
                                out=y2T[:, h * 4:(h + 1) * 4,
                                        t4 * 128:(t4 + 1) * 128],
                                in_=tp2[:])
                        nc.tensor.transpose(mr2T_ps[:, t4 * 128:(t4 + 1) * 128],
                                            mr2[:], ident[:])
                    mr2T = lnp2.tile([1, 512], bf16, tag="mr2T_sb")
                    nc.scalar.copy(out=mr2T[:], in_=mr2T_ps[:])

                    # fc1 + bias + gelu (fused on act) -> g1T resident
                    for ff in range(NF):
                        w1t = w1s.tile([128, D8, 128], bf16, tag="w1t")
                        nc.sync.dma_start(out=w1t[:], in_=w1_h[ff])
                        ps = ps_f1.tile([128, 512], f32, tag="f1")
                        for a in range(D8):
                            nc.tensor.matmul(ps[:], w1t[:, a, :], y2T[:, a, :],
                                             start=(a == 0), stop=False)
                        nc.tensor.matmul(ps[:], ns1_sb[:, ff * 128:(ff + 1) * 128],
                                         mr2T[:], start=False, stop=True)
                        nc.scalar.activation(out=g1T[:, ff, :], in_=ps[:], func=AF.Gelu,
                                             bias=b1_sb[:, ff:ff + 1])

                # fc2: 8 psum accumulators (4 token tiles x 2 column halves)
                ps_f2 = C.enter_context(tc.tile_pool(name="ps_f2", bufs=1, space="PSUM"))
                outp = C.enter_context(tc.tile_pool(name="outp", bufs=2))
                accs = [ps_f2.tile([128, 512], f32, name=f"acc{i}", tag=f"acc{i}")
                        for i in range(8)]
                for ff in range(NF):
                    w2t = w2sp.tile([128, D], bf16, tag="w2t")
                    nc.sync.dma_start(out=w2t[:], in_=w2_h[ff])
                    for t4 in range(4):
                        for dc in range(2):
                            nc.tensor.matmul(accs[t4 * 2 + dc][:],
                                             g1T[:, ff, t4 * 128:(t4 + 1) * 128],
                                             w2t[:, dc * 512:(dc + 1) * 512],
                                             start=(ff == 0), stop=False)
                for t4 in range(4):
                    ot = outp.tile([128, D], f32, tag="ot")
                    for dc in range(2):
                        i = t4 * 2 + dc
                        nc.tensor.matmul(accs[i][:], ones[:, 0:128],
                                         b2_sb[:, dc * 512:(dc + 1) * 512],
                                         start=False, stop=True)
                        nc.vector.tensor_add(ot[:, dc * 512:(dc + 1) * 512], accs[i][:],
                                             x2_sb[:, t4, dc * 512:(dc + 1) * 512])
                    nc.sync.dma_start(out=out_h[t4 * 128:(t4 + 1) * 128, :], in_=ot[:])

        def one_pass():
            with ExitStack() as P:
                rB = P.enter_context(tc.tile_pool(name="rB", bufs=1))
                with ExitStack() as A:
                    wA = A.enter_context(tc.tile_pool(name="wA", bufs=1))
                    QT = wA.tile([128, TOK], bf16)   # rows: (h2, hd)
                    KT = wA.tile([128, TOK], bf16)
                    # token-major V per 128-tok group g, per head:
                    # 64 dims + 64 ones columns (denominator rows of av)
                    V = wA.tile([128, TOK // 128, 2, 2 * HD], bf16)
                    phase_a1(wA, QT, KT, V)
                    # prefetch the proj weight during attention (act dma queue)
                    wp_sb = rB.tile([128, D8, D], bf16)
                    nc.scalar.dma_start(out=wp_sb[:], in_=wp_h[:])
                    pools = (
                        A.enter_context(tc.tile_pool(name="ptp", bufs=3)),
                        A.enter_context(tc.tile_pool(name="smp", bufs=2)),
                        A.enter_context(tc.tile_pool(name="aop", bufs=2)),
                        A.enter_context(tc.tile_pool(name="ps_s", bufs=3, space="PSUM")),
                        A.enter_context(tc.tile_pool(name="ps_av", bufs=2, space="PSUM")),
                    )
                    # head 0 of both batches, then its half-AllToAll overlaps
                    # head 1's attention; second half-AllToAll after.
                    phase_a2(pools, 0, Ic0, QT, KT, V)
                    nc.gpsimd.collective_compute(
                        "AllToAll", ALU.bypass,
                        replica_groups=[list(range(NCORES))],
                        ins=[Ic0[:]], outs=[Oc0[:]],
                    )
                    phase_a2(pools, 1, Ic1, QT, KT, V)

                # ----------- AllToAll (head 1): head-major -> token-major --------
                nc.gpsimd.collective_compute(
                    "AllToAll", ALU.bypass,
                    replica_groups=[list(range(NCORES))],
                    ins=[Ic1[:]], outs=[Oc1[:]],
                )

                phase_c(rB, wp_sb)

        for _ in range(loops):
            one_pass()
    nc.finalize()
    return nc


_NC_CACHE = []
LAST = None


def _get_nc():
    if not _NC_CACHE:
        _NC_CACHE.append(build())
    return _NC_CACHE[0]


def prepare_in_maps(inputs):
    from ml_dtypes import bfloat16

    f = np.float32
    x = np.asarray(inputs["x"], f).reshape(TOK, D)
    ln1_g = np.asarray(inputs["ln1_g"], np.float64)
    ln1_b = np.asarray(inputs["ln1_b"], np.float64)
    ln2_g = np.asarray(inputs["ln2_g"], np.float64)
    ln2_b = np.asarray(inputs["ln2_b"], np.float64)
    w_qkv = np.asarray(inputs["w_qkv"], np.float64)
    b_qkv = np.asarray(inputs["b_qkv"], np.float64)
    w_fc1 = np.asarray(inputs["w_fc1"], np.float64)
    b_fc1 = np.asarray(inputs["b_fc1"], np.float64)

    w_eff = (w_qkv * ln1_g[:, None]).astype(f)
    b_eff = (b_qkv + ln1_b @ w_qkv).astype(f)
    w1_eff = (w_fc1 * ln2_g[:, None]).astype(f)
    b1_eff = (b_fc1 + ln2_b @ w_fc1).astype(f)
    wp = np.asarray(inputs["w_proj"], f)
    bp = np.asarray(inputs["b_proj"], f).reshape(1, D)
    w2 = np.asarray(inputs["w_fc2"], f)
    b2 = np.asarray(inputs["b_fc2"], f).reshape(1, D)

    bf = lambda a: np.ascontiguousarray(a).astype(bfloat16)
    # [D, N] weight -> stationary layout [p, a, cols], feature f = a*128+p
    stat = lambda w: np.ascontiguousarray(
        w.reshape(D8, 128, w.shape[1]).transpose(1, 0, 2))

    x_bf = bf(x)
    wp_r = bf(stat(wp))                                     # [128, 8, 1024]
    w1_r = bf(np.ascontiguousarray(                         # [32, 128, 8, 128]
        w1_eff.reshape(D8, 128, NF, 128).transpose(2, 1, 0, 3)))
    w2_r = bf(w2.reshape(NF, 128, D))                       # [32, 128, 1024]
    b1_c = np.ascontiguousarray(b1_eff.reshape(NF, 128).T, dtype=f)  # [128, 32]
    ns1 = bf(-w1_eff.sum(axis=0).reshape(1, FF))            # [1, 4096]

    # additive causal mask for the diagonal 128x128 block of scores[k, q]:
    # valid (zero) where q >= k, else -1e9
    mask = np.full((128, 128), -1e9, f)
    for i in range(128):
        mask[i, i:] = 0.0
    ident = np.eye(128, dtype=bfloat16)

    in_maps = []
    for c in range(NCORES):
        cs = slice(128 * c, 128 * (c + 1))
        in_maps.append({
            "x": x_bf,
            "xc": np.ascontiguousarray(x[CHK * c:CHK * (c + 1)]),
            "wq": bf(stat(w_eff[:, 0 * D:1 * D][:, cs])),
            "wk": bf(stat(w_eff[:, 1 * D:2 * D][:, cs])),
            "wv": bf(stat(w_eff[:, 2 * D:3 * D][:, cs])),
            "bq": np.ascontiguousarray(b_eff[0 * D:1 * D][cs].reshape(128, 1), dtype=f),
            "bk": np.ascontiguousarray(b_eff[1 * D:2 * D][cs].reshape(128, 1), dtype=f),
            "bv": bf(b_eff[2 * D:3 * D][cs].reshape(1, 128)),
            "nsq": bf(-w_eff[:, 0 * D:1 * D][:, cs].sum(axis=0).reshape(1, 128)),
            "nsk": bf(-w_eff[:, 1 * D:2 * D][:, cs].sum(axis=0).reshape(1, 128)),
            "nsv": bf(-w_eff[:, 2 * D:3 * D][:, cs].sum(axis=0).reshape(1, 128)),
            "ns1": ns1,
            "wp": wp_r, "bp": bf(bp),
            "w1": w1_r, "b1": b1_c,
            "w2": w2_r, "b2": bf(b2),
            "mask": mask, "ident": ident,
            "ones": np.ones((1, 512), bfloat16),
            "onesv": np.ones((128, TOK // 128, 2, HD), bfloat16),
        })
    return in_maps


def kernel(**inputs):
    global LAST
    in_maps = prepare_in_maps(inputs)
    nc = _get_nc()
    res = run_bass_kernel_spmd(nc, in_maps, list(range(NCORES)))
    LAST = res
    out = np.concatenate([res.results[c]["out"] for c in range(NCORES)], axis=0)
    return out.reshape(B, S, D).astype(np.float32, copy=False)
